# revision 4
# baseline (speedup 1.0000x reference)
"""LoRALinear Trainium2 kernel.

out[b] = x[b] @ W.T + bias + (x[b] @ A[id_b]) @ B[id_b]

Strategy (8 NeuronCores, data-parallel over batch):
- Core b handles batch b (2048 tokens). The per-batch LoRA adapter is
  gathered on the host (lora_int_id is host-visible), so each core only
  reads its own A/B stack.
- All matmul operands are pre-cast to bf16 and pre-tiled on the host into
  DMA-friendly layouts (contiguous >=4KB runs per SBUF partition).
- Kernel computes out^T[OUT, S] weight-stationary: for each 128-row output
  tile, accumulate 32 K-tiles of W @ x^T into PSUM (f32), then fold the
  LoRA delta AND the bias in as one extra K=65 accumulation step using
  [B_sel; bias]^T @ [temp^T; ones] before draining PSUM to SBUF/DRAM.
- temp^T = A_sel^T @ x^T is computed once per core in a small pre-pass.
- Host transposes each core's out^T back to [S, OUT].
"""

import os

import numpy as np
import ml_dtypes

P = 128
B, S, IN, OUT = 8, 2048, 4096, 4096
RANK = 64
N_CORES = 8

_NC_CACHE = {}
LAST_EXEC_TIME_NS = None


def _build(s, in_f, out_f, rank, f_chunk):
    """Build + compile the single-core Bass program (same NEFF for all cores)."""
    import concourse.tile as tile
    from concourse import bacc, mybir

    kt_n = in_f // P
    mt_n = out_f // P
    nf = s // f_chunk
    bf = mybir.dt.bfloat16
    f32 = mybir.dt.float32

    nc = bacc.Bacc("TRN2", target_bir_lowering=False, debug=False)
    xt = nc.dram_tensor("xt", [P, kt_n * s], bf, kind="ExternalInput")
    w = nc.dram_tensor("w", [mt_n, P, kt_n * P], bf, kind="ExternalInput")
    a = nc.dram_tensor("a", [P, kt_n * rank], bf, kind="ExternalInput")
    baug = nc.dram_tensor("baug", [rank + 1, out_f], bf, kind="ExternalInput")
    out_t = nc.dram_tensor("out_t", [out_f, s], f32, kind="ExternalOutput")

    with tile.TileContext(nc) as tc:
        with (
            tc.tile_pool(name="xp", bufs=1) as xp,
            tc.tile_pool(name="cst", bufs=1) as cst,
            tc.tile_pool(name="wp", bufs=3) as wpool,
            tc.tile_pool(name="op", bufs=2) as opool,
            tc.tile_pool(name="mp", bufs=2, space="PSUM") as mpool,
        ):
            x_sb = xp.tile([P, kt_n * s], bf)
            for kt in range(kt_n):
                nc.sync.dma_start(x_sb[:, kt * s : (kt + 1) * s], xt[:, kt * s : (kt + 1) * s])
            a_sb = cst.tile([P, kt_n * rank], bf)
            nc.sync.dma_start(a_sb[:], a[:])
            b_sb = cst.tile([rank + 1, out_f], bf)
            nc.sync.dma_start(b_sb[:], baug[:])
            t_sb = cst.tile([rank + 1, s], bf)
            nc.vector.memset(t_sb[rank : rank + 1, :], 1.0)

            # pre-pass: temp^T[r, t] = sum_k A[k, r] * x^T[k, t]
            pt = mpool.tile([rank, s], f32, tag="ps")
            for kt in range(kt_n):
                for f in range(nf):
                    nc.tensor.matmul(
                        pt[:, f * f_chunk : (f + 1) * f_chunk],
                        a_sb[:, kt * rank : (kt + 1) * rank],
                        x_sb[:, kt * s + f * f_chunk : kt * s + (f + 1) * f_chunk],
                        start=(kt == 0),
                        stop=(kt == kt_n - 1),
                    )
            nc.vector.tensor_copy(t_sb[0:rank, :], pt[:])

            for mt in range(mt_n):
                w_sb = wpool.tile([P, kt_n * P], bf)
                nc.sync.dma_start(w_sb[:], w[mt, :, :])
                ps = mpool.tile([P, s], f32, tag="ps")
                for kt in range(kt_n):
                    for f in range(nf):
                        nc.tensor.matmul(
                            ps[:, f * f_chunk : (f + 1) * f_chunk],
                            w_sb[:, kt * P : (kt + 1) * P],
                            x_sb[:, kt * s + f * f_chunk : kt * s + (f + 1) * f_chunk],
                            start=(kt == 0),
                            stop=False,
                        )
                # LoRA delta + bias: one K=rank+1 accumulation step
                for f in range(nf):
                    nc.tensor.matmul(
                        ps[:, f * f_chunk : (f + 1) * f_chunk],
                        b_sb[:, mt * P : (mt + 1) * P],
                        t_sb[:, f * f_chunk : (f + 1) * f_chunk],
                        start=False,
                        stop=True,
                    )
                ob = opool.tile([P, s], f32)
                nc.vector.tensor_copy(ob[:], ps[:])
                nc.sync.dma_start(out_t[mt * P : (mt + 1) * P, :], ob[:])

    nc.compile()
    return nc


def _get_nc(s=S, in_f=IN, out_f=OUT, rank=RANK, f_chunk=512):
    key = (s, in_f, out_f, rank, f_chunk)
    if key not in _NC_CACHE:
        _NC_CACHE[key] = _build(*key)
    return _NC_CACHE[key]


def _prep_in_maps(x, ids, weight, bias, lora_a, lora_b):
    """Host-side shard + tile + bf16 cast. Returns one in_map per core."""
    bf16 = ml_dtypes.bfloat16
    s, in_f = x.shape[1], x.shape[2]
    out_f = weight.shape[0]
    rank = lora_a.shape[2]
    kt_n = in_f // P
    mt_n = out_f // P

    # weight -> [mt, kp, kt, mj] bf16 (lhsT panels, contiguous per partition)
    wt = (
        weight.reshape(mt_n, P, kt_n, P)
        .transpose(0, 3, 2, 1)
        .astype(bf16)
        .reshape(mt_n, P, kt_n * P)
    )
    wt = np.ascontiguousarray(wt)

    in_maps = []
    for b in range(x.shape[0]):
        xt = (
            x[b]
            .reshape(s, kt_n, P)
            .transpose(2, 1, 0)
            .astype(bf16)
            .reshape(P, kt_n * s)
        )
        at = (
            lora_a[ids[b]]
            .reshape(kt_n, P, rank)
            .transpose(1, 0, 2)
            .astype(bf16)
            .reshape(P, kt_n * rank)
        )
        baug = np.concatenate([lora_b[ids[b]], bias[None, :]], axis=0).astype(bf16)
        in_maps.append(
            {
                "xt": np.ascontiguousarray(xt),
                "w": wt,
                "a": np.ascontiguousarray(at),
                "baug": np.ascontiguousarray(baug),
            }
        )
    return in_maps


def _install_ntff_hook():
    """Provide antenv.axon_hooks so run_bass_kernel_spmd(trace=True) can ship
    NTFF profiles back through the axon tunnel (missing in this image)."""
    import sys
    import types

    if "antenv.axon_hooks" in sys.modules:
        return
    from trn_agent_boot.trn_boot import _ntff_profile_via_ctypes

    hook = _ntff_profile_via_ctypes("/opt/axon/libaxon_pjrt.so")
    if hook is None:
        raise RuntimeError("no profiling symbols in libaxon_pjrt.so")
    mod = types.ModuleType("antenv.axon_hooks")
    mod.get_axon_ntff_profile_hook = lambda: hook
    mod.set_axon_ntff_profile_hook = lambda h: None
    sys.modules["antenv.axon_hooks"] = mod


def kernel(x, lora_int_id, weight, bias, lora_a_weights, lora_b_weights):
    global LAST_EXEC_TIME_NS
    from concourse.bass_utils import run_bass_kernel_spmd

    x = np.asarray(x, dtype=np.float32)
    ids = np.asarray(lora_int_id).astype(np.int64)
    weight = np.asarray(weight, dtype=np.float32)
    bias = np.asarray(bias, dtype=np.float32)
    lora_a = np.asarray(lora_a_weights, dtype=np.float32)
    lora_b = np.asarray(lora_b_weights, dtype=np.float32)

    nc = _get_nc()
    in_maps = _prep_in_maps(x, ids, weight, bias, lora_a, lora_b)

    trace = bool(int(os.environ.get("KERNEL_TRACE", "0")))
    if trace:
        try:
            _install_ntff_hook()
        except Exception:
            trace = False

    res = run_bass_kernel_spmd(nc, in_maps, core_ids=list(range(N_CORES)), trace=trace)
    LAST_EXEC_TIME_NS = res.exec_time_ns

    out = np.empty((x.shape[0], x.shape[1], weight.shape[0]), dtype=np.float32)
    for b in range(x.shape[0]):
        out[b] = res.results[b]["out_t"].T
    return out


# revision 8
# speedup vs baseline: 1.0288x; 1.0288x over previous
"""LoRALinear Trainium2 kernel.

out[b] = x[b] @ W.T + bias + (x[b] @ A[id_b]) @ B[id_b]

Strategy (8 NeuronCores, data-parallel over batch):
- Core b handles batch b (2048 tokens). The per-batch LoRA adapter is
  gathered on the host (lora_int_id is host-visible), so each core only
  reads its own A/B stack.
- All matmul operands are pre-cast to bf16 and pre-tiled on the host into
  DMA-friendly layouts (contiguous >=4KB runs per SBUF partition).
- Kernel computes out^T[OUT, S] weight-stationary: for each 128-row output
  tile, accumulate 32 K-tiles of W @ x^T into PSUM (f32), then fold the
  LoRA delta AND the bias in as one extra K=65 accumulation step using
  [B_sel; bias]^T @ [temp^T; ones] before draining PSUM to SBUF/DRAM.
- temp^T = A_sel^T @ x^T is computed once per core in a small pre-pass.
- Host transposes each core's out^T back to [S, OUT].
"""

import os

import numpy as np
import ml_dtypes

P = 128
B, S, IN, OUT = 8, 2048, 4096, 4096
RANK = 64
N_CORES = 8

_NC_CACHE = {}
LAST_EXEC_TIME_NS = None
LAST_RESULT = None


def _build(s, in_f, out_f, rank, f_chunk):
    """Build + compile the single-core Bass program (same NEFF for all cores)."""
    import concourse.tile as tile
    from concourse import bacc, mybir

    kt_n = in_f // P
    mt_n = out_f // P
    nf = s // f_chunk
    bf = mybir.dt.bfloat16
    f32 = mybir.dt.float32

    nc = bacc.Bacc("TRN2", target_bir_lowering=False, debug=False)
    xt = nc.dram_tensor("xt", [P, kt_n * s], bf, kind="ExternalInput")
    w = nc.dram_tensor("w", [mt_n, P, kt_n * P], bf, kind="ExternalInput")
    a = nc.dram_tensor("a", [P, kt_n * rank], bf, kind="ExternalInput")
    baug = nc.dram_tensor("baug", [rank + 1, out_f], bf, kind="ExternalInput")
    out_t = nc.dram_tensor("out_t", [out_f, s], f32, kind="ExternalOutput")

    with tile.TileContext(nc) as tc:
        with (
            tc.tile_pool(name="xp", bufs=1) as xp,
            tc.tile_pool(name="cst", bufs=1) as cst,
            tc.tile_pool(name="wp", bufs=3) as wpool,
            tc.tile_pool(name="op", bufs=2) as opool,
            tc.tile_pool(name="mp", bufs=2, space="PSUM") as mpool,
        ):
            # DMA order on the sync (SP-HWDGE) queue: small constants + first
            # weight panel BEFORE the big x stream, so PE can start ~6us in
            # and trail the x DMA chunk-by-chunk. Output DMAs go on the
            # scalar (ACT-HWDGE) queue so they never block weight prefetch.
            a_sb = cst.tile([P, kt_n * rank], bf)
            nc.sync.dma_start(a_sb[:], a[:])
            b_sb = cst.tile([rank + 1, out_f], bf)
            nc.sync.dma_start(b_sb[:], baug[:])
            w_sb0 = wpool.tile([P, kt_n * P], bf, tag="w")
            nc.sync.dma_start(w_sb0[:], w[0, :, :])
            x_sb = xp.tile([P, kt_n * s], bf)
            for kt in range(kt_n):
                nc.sync.dma_start(x_sb[:, kt * s : (kt + 1) * s], xt[:, kt * s : (kt + 1) * s])
            t_sb = cst.tile([rank + 1, s], bf)
            nc.vector.memset(t_sb[rank : rank + 1, :], 1.0)

            def k_loop(ps, w_sb, kt, stop_last=False):
                for f in range(nf):
                    nc.tensor.matmul(
                        ps[:, f * f_chunk : (f + 1) * f_chunk],
                        w_sb[:, kt * P : (kt + 1) * P],
                        x_sb[:, kt * s + f * f_chunk : kt * s + (f + 1) * f_chunk],
                        start=(kt == 0),
                        stop=stop_last and (kt == kt_n - 1),
                    )

            def delta_and_drain(ps, mt):
                # LoRA delta + bias: one K=rank+1 accumulation step
                for f in range(nf):
                    nc.tensor.matmul(
                        ps[:, f * f_chunk : (f + 1) * f_chunk],
                        b_sb[:, mt * P : (mt + 1) * P],
                        t_sb[:, f * f_chunk : (f + 1) * f_chunk],
                        start=False,
                        stop=True,
                    )
                ob = opool.tile([P, s], f32, tag="ob")
                nc.vector.tensor_copy(ob[:], ps[:])
                nc.scalar.dma_start(out_t[mt * P : (mt + 1) * P, :], ob[:])

            # ramp: pre-pass (temp^T = A^T @ x^T) interleaved with mt=0's
            # k-loop, both trailing the x DMA chunk-by-chunk.
            pt = mpool.tile([rank, s], f32, tag="ps")
            ps0 = mpool.tile([P, s], f32, tag="ps")
            for kt in range(kt_n):
                for f in range(nf):
                    nc.tensor.matmul(
                        pt[:, f * f_chunk : (f + 1) * f_chunk],
                        a_sb[:, kt * rank : (kt + 1) * rank],
                        x_sb[:, kt * s + f * f_chunk : kt * s + (f + 1) * f_chunk],
                        start=(kt == 0),
                        stop=(kt == kt_n - 1),
                    )
                k_loop(ps0, w_sb0, kt)
            for f in range(nf):
                nc.vector.tensor_copy(
                    t_sb[0:rank, f * f_chunk : (f + 1) * f_chunk],
                    pt[:, f * f_chunk : (f + 1) * f_chunk],
                )
            delta_and_drain(ps0, 0)

            for mt in range(1, mt_n):
                w_sb = wpool.tile([P, kt_n * P], bf, tag="w")
                nc.sync.dma_start(w_sb[:], w[mt, :, :])
                ps = mpool.tile([P, s], f32, tag="ps")
                for kt in range(kt_n):
                    k_loop(ps, w_sb, kt)
                delta_and_drain(ps, mt)

    nc.compile()
    return nc


def _get_nc(s=S, in_f=IN, out_f=OUT, rank=RANK, f_chunk=512):
    key = (s, in_f, out_f, rank, f_chunk)
    if key not in _NC_CACHE:
        _NC_CACHE[key] = _build(*key)
    return _NC_CACHE[key]


def _prep_in_maps(x, ids, weight, bias, lora_a, lora_b):
    """Host-side shard + tile + bf16 cast. Returns one in_map per core."""
    bf16 = ml_dtypes.bfloat16
    s, in_f = x.shape[1], x.shape[2]
    out_f = weight.shape[0]
    rank = lora_a.shape[2]
    kt_n = in_f // P
    mt_n = out_f // P

    # weight -> [mt, kp, kt, mj] bf16 (lhsT panels, contiguous per partition)
    wt = (
        weight.reshape(mt_n, P, kt_n, P)
        .transpose(0, 3, 2, 1)
        .astype(bf16)
        .reshape(mt_n, P, kt_n * P)
    )
    wt = np.ascontiguousarray(wt)

    in_maps = []
    for b in range(x.shape[0]):
        xt = (
            x[b]
            .reshape(s, kt_n, P)
            .transpose(2, 1, 0)
            .astype(bf16)
            .reshape(P, kt_n * s)
        )
        at = (
            lora_a[ids[b]]
            .reshape(kt_n, P, rank)
            .transpose(1, 0, 2)
            .astype(bf16)
            .reshape(P, kt_n * rank)
        )
        baug = np.concatenate([lora_b[ids[b]], bias[None, :]], axis=0).astype(bf16)
        in_maps.append(
            {
                "xt": np.ascontiguousarray(xt),
                "w": wt,
                "a": np.ascontiguousarray(at),
                "baug": np.ascontiguousarray(baug),
            }
        )
    return in_maps


def _install_ntff_hook():
    """Provide antenv.axon_hooks so run_bass_kernel_spmd(trace=True) can ship
    NTFF profiles back through the axon tunnel (missing in this image)."""
    import sys
    import types

    if "antenv.axon_hooks" in sys.modules:
        return
    from trn_agent_boot.trn_boot import _ntff_profile_via_ctypes

    hook = _ntff_profile_via_ctypes("/opt/axon/libaxon_pjrt.so")
    if hook is None:
        raise RuntimeError("no profiling symbols in libaxon_pjrt.so")
    mod = types.ModuleType("antenv.axon_hooks")
    mod.get_axon_ntff_profile_hook = lambda: hook
    mod.set_axon_ntff_profile_hook = lambda h: None
    sys.modules["antenv.axon_hooks"] = mod


def kernel(x, lora_int_id, weight, bias, lora_a_weights, lora_b_weights):
    global LAST_EXEC_TIME_NS
    from concourse.bass_utils import run_bass_kernel_spmd

    x = np.asarray(x, dtype=np.float32)
    ids = np.asarray(lora_int_id).astype(np.int64)
    weight = np.asarray(weight, dtype=np.float32)
    bias = np.asarray(bias, dtype=np.float32)
    lora_a = np.asarray(lora_a_weights, dtype=np.float32)
    lora_b = np.asarray(lora_b_weights, dtype=np.float32)

    nc = _get_nc()
    in_maps = _prep_in_maps(x, ids, weight, bias, lora_a, lora_b)

    trace = bool(int(os.environ.get("KERNEL_TRACE", "0")))
    if trace:
        try:
            _install_ntff_hook()
        except Exception:
            trace = False

    res = run_bass_kernel_spmd(nc, in_maps, core_ids=list(range(N_CORES)), trace=trace)
    LAST_EXEC_TIME_NS = res.exec_time_ns
    global LAST_RESULT
    LAST_RESULT = res

    out = np.empty((x.shape[0], x.shape[1], weight.shape[0]), dtype=np.float32)
    for b in range(x.shape[0]):
        out[b] = res.results[b]["out_t"].T
    return out


# revision 10
# speedup vs baseline: 1.0291x; 1.0002x over previous
"""LoRALinear Trainium2 kernel.

out[b] = x[b] @ W.T + bias + (x[b] @ A[id_b]) @ B[id_b]

Strategy (8 NeuronCores, data-parallel over batch):
- Core b handles batch b (2048 tokens). The per-batch LoRA adapter is
  gathered on the host (lora_int_id is host-visible), so each core only
  reads its own A/B stack.
- All matmul operands are pre-cast to bf16 and pre-tiled on the host into
  DMA-friendly layouts (contiguous >=4KB runs per SBUF partition).
- Kernel computes out^T[OUT, S] weight-stationary: for each 128-row output
  tile, accumulate 32 K-tiles of W @ x^T into PSUM (f32), then fold the
  LoRA delta AND the bias in as one extra K=65 accumulation step using
  [B_sel; bias]^T @ [temp^T; ones] before draining PSUM to SBUF/DRAM.
- temp^T = A_sel^T @ x^T is computed once per core in a small pre-pass.
- Host transposes each core's out^T back to [S, OUT].
"""

import os

import numpy as np
import ml_dtypes

P = 128
B, S, IN, OUT = 8, 2048, 4096, 4096
RANK = 64
N_CORES = 8

_NC_CACHE = {}
LAST_EXEC_TIME_NS = None
LAST_RESULT = None


def _build(s, in_f, out_f, rank, f_chunk):
    """Build + compile the single-core Bass program (same NEFF for all cores)."""
    import concourse.tile as tile
    from concourse import bacc, mybir

    kt_n = in_f // P
    mt_n = out_f // P
    nf = s // f_chunk
    bf = mybir.dt.bfloat16
    f32 = mybir.dt.float32

    nc = bacc.Bacc("TRN2", target_bir_lowering=False, debug=False)
    xt = nc.dram_tensor("xt", [P, kt_n * s], bf, kind="ExternalInput")
    w = nc.dram_tensor("w", [mt_n, P, kt_n * P], bf, kind="ExternalInput")
    a = nc.dram_tensor("a", [P, kt_n * rank], bf, kind="ExternalInput")
    baug = nc.dram_tensor("baug", [rank + 1, out_f], bf, kind="ExternalInput")
    out_t = nc.dram_tensor("out_t", [out_f, s], f32, kind="ExternalOutput")

    with tile.TileContext(nc) as tc:
        with (
            tc.tile_pool(name="xp", bufs=1) as xp,
            tc.tile_pool(name="cst", bufs=1) as cst,
            tc.tile_pool(name="wp", bufs=3) as wpool,
            tc.tile_pool(name="op", bufs=2) as opool,
            tc.tile_pool(name="mp", bufs=2, space="PSUM") as mpool,
        ):
            # DMA order on the sync (SP-HWDGE) queue: small constants + first
            # weight panel BEFORE the big x stream, so PE can start ~6us in
            # and trail the x DMA chunk-by-chunk. Output DMAs go on the
            # scalar (ACT-HWDGE) queue so they never block weight prefetch.
            a_sb = cst.tile([P, kt_n * rank], bf)
            nc.sync.dma_start(a_sb[:], a[:])
            x_sb = xp.tile([P, kt_n * s], bf)
            b_sb = cst.tile([rank + 1, out_f], bf)
            w_sb0 = wpool.tile([P, kt_n * P], bf, tag="w")
            for kt in range(kt_n):
                nc.sync.dma_start(x_sb[:, kt * s : (kt + 1) * s], xt[:, kt * s : (kt + 1) * s])
                if kt == 0:
                    # first weight panel right after the first x chunk: PE can
                    # start at ~9us and trail the x stream chunk-by-chunk
                    nc.sync.dma_start(w_sb0[:], w[0, :, :])
                elif kt == 8:
                    # B/bias panel not needed until the first delta (~70us)
                    nc.sync.dma_start(b_sb[:], baug[:])
            t_sb = cst.tile([rank + 1, s], bf)
            nc.vector.memset(t_sb[rank : rank + 1, :], 1.0)

            def k_loop(ps, w_sb, kt, stop_last=False):
                for f in range(nf):
                    nc.tensor.matmul(
                        ps[:, f * f_chunk : (f + 1) * f_chunk],
                        w_sb[:, kt * P : (kt + 1) * P],
                        x_sb[:, kt * s + f * f_chunk : kt * s + (f + 1) * f_chunk],
                        start=(kt == 0),
                        stop=stop_last and (kt == kt_n - 1),
                    )

            def delta_and_drain(ps, mt):
                # LoRA delta + bias: one K=rank+1 accumulation step, then
                # drain + store per f-chunk so the copy/DMA of chunk f
                # overlaps the delta matmul of chunk f+1.
                ob = opool.tile([P, s], f32, tag="ob")
                for f in range(nf):
                    fs = slice(f * f_chunk, (f + 1) * f_chunk)
                    nc.tensor.matmul(
                        ps[:, fs],
                        b_sb[:, mt * P : (mt + 1) * P],
                        t_sb[:, fs],
                        start=False,
                        stop=True,
                    )
                    nc.vector.tensor_copy(ob[:, fs], ps[:, fs])
                    nc.scalar.dma_start(out_t[mt * P : (mt + 1) * P, fs], ob[:, fs])

            # ramp: pre-pass (temp^T = A^T @ x^T) interleaved with mt=0's
            # k-loop, both trailing the x DMA chunk-by-chunk.
            pt = mpool.tile([rank, s], f32, tag="ps")
            ps0 = mpool.tile([P, s], f32, tag="ps")
            for kt in range(kt_n):
                for f in range(nf):
                    nc.tensor.matmul(
                        pt[:, f * f_chunk : (f + 1) * f_chunk],
                        a_sb[:, kt * rank : (kt + 1) * rank],
                        x_sb[:, kt * s + f * f_chunk : kt * s + (f + 1) * f_chunk],
                        start=(kt == 0),
                        stop=(kt == kt_n - 1),
                    )
                k_loop(ps0, w_sb0, kt)
            for f in range(nf):
                nc.vector.tensor_copy(
                    t_sb[0:rank, f * f_chunk : (f + 1) * f_chunk],
                    pt[:, f * f_chunk : (f + 1) * f_chunk],
                )
            delta_and_drain(ps0, 0)

            for mt in range(1, mt_n):
                w_sb = wpool.tile([P, kt_n * P], bf, tag="w")
                nc.sync.dma_start(w_sb[:], w[mt, :, :])
                ps = mpool.tile([P, s], f32, tag="ps")
                for kt in range(kt_n):
                    k_loop(ps, w_sb, kt)
                delta_and_drain(ps, mt)

    nc.compile()
    return nc


def _get_nc(s=S, in_f=IN, out_f=OUT, rank=RANK, f_chunk=512):
    key = (s, in_f, out_f, rank, f_chunk)
    if key not in _NC_CACHE:
        _NC_CACHE[key] = _build(*key)
    return _NC_CACHE[key]


def _prep_in_maps(x, ids, weight, bias, lora_a, lora_b):
    """Host-side shard + tile + bf16 cast. Returns one in_map per core."""
    bf16 = ml_dtypes.bfloat16
    s, in_f = x.shape[1], x.shape[2]
    out_f = weight.shape[0]
    rank = lora_a.shape[2]
    kt_n = in_f // P
    mt_n = out_f // P

    # weight -> [mt, kp, kt, mj] bf16 (lhsT panels, contiguous per partition)
    wt = (
        weight.reshape(mt_n, P, kt_n, P)
        .transpose(0, 3, 2, 1)
        .astype(bf16)
        .reshape(mt_n, P, kt_n * P)
    )
    wt = np.ascontiguousarray(wt)

    in_maps = []
    for b in range(x.shape[0]):
        xt = (
            x[b]
            .reshape(s, kt_n, P)
            .transpose(2, 1, 0)
            .astype(bf16)
            .reshape(P, kt_n * s)
        )
        at = (
            lora_a[ids[b]]
            .reshape(kt_n, P, rank)
            .transpose(1, 0, 2)
            .astype(bf16)
            .reshape(P, kt_n * rank)
        )
        baug = np.concatenate([lora_b[ids[b]], bias[None, :]], axis=0).astype(bf16)
        in_maps.append(
            {
                "xt": np.ascontiguousarray(xt),
                "w": wt,
                "a": np.ascontiguousarray(at),
                "baug": np.ascontiguousarray(baug),
            }
        )
    return in_maps


def _install_ntff_hook():
    """Provide antenv.axon_hooks so run_bass_kernel_spmd(trace=True) can ship
    NTFF profiles back through the axon tunnel (missing in this image)."""
    import sys
    import types

    if "antenv.axon_hooks" in sys.modules:
        return
    from trn_agent_boot.trn_boot import _ntff_profile_via_ctypes

    hook = _ntff_profile_via_ctypes("/opt/axon/libaxon_pjrt.so")
    if hook is None:
        raise RuntimeError("no profiling symbols in libaxon_pjrt.so")
    mod = types.ModuleType("antenv.axon_hooks")
    mod.get_axon_ntff_profile_hook = lambda: hook
    mod.set_axon_ntff_profile_hook = lambda h: None
    sys.modules["antenv.axon_hooks"] = mod


def kernel(x, lora_int_id, weight, bias, lora_a_weights, lora_b_weights):
    global LAST_EXEC_TIME_NS
    from concourse.bass_utils import run_bass_kernel_spmd

    x = np.asarray(x, dtype=np.float32)
    ids = np.asarray(lora_int_id).astype(np.int64)
    weight = np.asarray(weight, dtype=np.float32)
    bias = np.asarray(bias, dtype=np.float32)
    lora_a = np.asarray(lora_a_weights, dtype=np.float32)
    lora_b = np.asarray(lora_b_weights, dtype=np.float32)

    nc = _get_nc()
    in_maps = _prep_in_maps(x, ids, weight, bias, lora_a, lora_b)

    trace = bool(int(os.environ.get("KERNEL_TRACE", "0")))
    if trace:
        try:
            _install_ntff_hook()
        except Exception:
            trace = False

    res = run_bass_kernel_spmd(nc, in_maps, core_ids=list(range(N_CORES)), trace=trace)
    LAST_EXEC_TIME_NS = res.exec_time_ns
    global LAST_RESULT
    LAST_RESULT = res

    out = np.empty((x.shape[0], x.shape[1], weight.shape[0]), dtype=np.float32)
    for b in range(x.shape[0]):
        out[b] = res.results[b]["out_t"].T
    return out


# revision 13
# speedup vs baseline: 1.0360x; 1.0068x over previous
"""LoRALinear Trainium2 kernel.

out[b] = x[b] @ W.T + bias + (x[b] @ A[id_b]) @ B[id_b]

Strategy (8 NeuronCores, data-parallel over batch):
- Core b handles batch b (2048 tokens). The per-batch LoRA adapter is
  gathered on the host (lora_int_id is host-visible), so each core only
  reads its own A/B stack.
- All matmul operands are pre-cast to bf16 and pre-tiled on the host into
  DMA-friendly layouts (contiguous >=4KB runs per SBUF partition).
- Kernel computes out^T[OUT, S] weight-stationary: for each 128-row output
  tile, accumulate 32 K-tiles of W @ x^T into PSUM (f32), then fold the
  LoRA delta AND the bias in as one extra K=65 accumulation step using
  [B_sel; bias]^T @ [temp^T; ones] before draining PSUM to SBUF/DRAM.
- temp^T = A_sel^T @ x^T is computed once per core in a small pre-pass.
- Host transposes each core's out^T back to [S, OUT].
"""

import os

import numpy as np
import ml_dtypes

P = 128
B, S, IN, OUT = 8, 2048, 4096, 4096
RANK = 64
N_CORES = 8

_NC_CACHE = {}
LAST_EXEC_TIME_NS = None
LAST_RESULT = None


def _build(s, in_f, out_f, rank, f_chunk):
    """Build + compile the single-core Bass program (same NEFF for all cores)."""
    import concourse.tile as tile
    from concourse import bacc, mybir

    kt_n = in_f // P
    mt_n = out_f // P
    nf = s // f_chunk
    bf = mybir.dt.bfloat16
    f32 = mybir.dt.float32

    nc = bacc.Bacc("TRN2", target_bir_lowering=False, debug=False)
    xt = nc.dram_tensor("xt", [P, kt_n * s], bf, kind="ExternalInput")
    w = nc.dram_tensor("w", [mt_n, P, kt_n * P], bf, kind="ExternalInput")
    a = nc.dram_tensor("a", [P, kt_n * rank], bf, kind="ExternalInput")
    baug = nc.dram_tensor("baug", [rank + 1, out_f], bf, kind="ExternalInput")
    out_t = nc.dram_tensor("out_t", [out_f, s], f32, kind="ExternalOutput")

    with tile.TileContext(nc) as tc:
        with (
            tc.tile_pool(name="xp", bufs=1) as xp,
            tc.tile_pool(name="cst", bufs=1) as cst,
            tc.tile_pool(name="wp", bufs=3) as wpool,
            tc.tile_pool(name="op", bufs=2) as opool,
            tc.tile_pool(name="mp", bufs=2, space="PSUM") as mpool,
        ):
            # DMA order on the sync (SP-HWDGE) queue: small constants + first
            # weight panel BEFORE the big x stream, so PE can start ~6us in
            # and trail the x DMA chunk-by-chunk. Output DMAs go on the
            # scalar (ACT-HWDGE) queue so they never block weight prefetch.
            a_sb = cst.tile([P, kt_n * rank], bf)
            nc.sync.dma_start(a_sb[:], a[:])
            x_sb = xp.tile([P, kt_n * s], bf)
            b_sb = cst.tile([rank + 1, out_f], bf)
            w_sb0 = wpool.tile([P, kt_n * P], bf, tag="w")
            for kt in range(kt_n):
                nc.sync.dma_start(x_sb[:, kt * s : (kt + 1) * s], xt[:, kt * s : (kt + 1) * s])
                if kt == 0:
                    # first weight panel right after the first x chunk: PE can
                    # start at ~9us and trail the x stream chunk-by-chunk
                    nc.sync.dma_start(w_sb0[:], w[0, :, :])
                elif kt == min(8, kt_n - 1):
                    # B/bias panel not needed until the first delta (~70us)
                    nc.sync.dma_start(b_sb[:], baug[:])
            t_sb = cst.tile([rank + 1, s], bf)
            nc.vector.memset(t_sb[rank : rank + 1, :], 1.0)
            ts2 = cst.tile([2 * rank, s - (nf // 2) * f_chunk], bf)

            def k_loop(ps, w_sb, kt, stop_last=False):
                for f in range(nf):
                    nc.tensor.matmul(
                        ps[:, f * f_chunk : (f + 1) * f_chunk],
                        w_sb[:, kt * P : (kt + 1) * P],
                        x_sb[:, kt * s + f * f_chunk : kt * s + (f + 1) * f_chunk],
                        start=(kt == 0),
                        stop=stop_last and (kt == kt_n - 1),
                    )

            def delta_and_drain(ps, mt):
                # LoRA delta + bias: one K=rank+1 accumulation step, then
                # drain + store per f-chunk so the copy/DMA of chunk f
                # overlaps the delta matmul of chunk f+1.
                ob = opool.tile([P, s], f32, tag="ob")
                for f in range(nf):
                    fs = slice(f * f_chunk, (f + 1) * f_chunk)
                    nc.tensor.matmul(
                        ps[:, fs],
                        b_sb[:, mt * P : (mt + 1) * P],
                        t_sb[:, fs],
                        start=False,
                        stop=True,
                    )
                    nc.vector.tensor_copy(ob[:, fs], ps[:, fs])
                    nc.scalar.dma_start(out_t[mt * P : (mt + 1) * P, fs], ob[:, fs])

            # ramp: pre-pass (temp^T = A^T @ x^T) interleaved with mt=0's
            # k-loop, both trailing the x DMA chunk-by-chunk. The pre-pass
            # is col-packed: M=64 uses only half the PE column groups, so
            # token-half 0 runs on PSUM partitions 0:64 and token-half 1 on
            # 64:128 concurrently (distinct col groups, distinct banks).
            assert nf % 2 == 0 and 2 * rank == P
            nh = nf // 2
            sh = nh * f_chunk
            pt = mpool.tile([2 * rank, s], f32, tag="ps")
            ps0 = mpool.tile([P, s], f32, tag="ps")
            for kt in range(kt_n):
                for fh in range(nh):
                    for half, f in ((0, fh), (1, fh + nh)):
                        nc.tensor.matmul(
                            pt[half * rank : (half + 1) * rank, f * f_chunk : (f + 1) * f_chunk],
                            a_sb[:, kt * rank : (kt + 1) * rank],
                            x_sb[:, kt * s + f * f_chunk : kt * s + (f + 1) * f_chunk],
                            start=(kt == 0),
                            stop=(kt == kt_n - 1),
                        )
                k_loop(ps0, w_sb0, kt)
            # assemble temp^T (+ones row already set) on partitions 0:64;
            # token-half 1 lands on partitions 64:128 so it bounces through
            # ts2 + an SBUF->SBUF DMA (engines can't move across partitions).
            for fh in range(nh):
                f = fh + nh
                nc.vector.tensor_copy(
                    t_sb[0:rank, fh * f_chunk : (fh + 1) * f_chunk],
                    pt[0:rank, fh * f_chunk : (fh + 1) * f_chunk],
                )
                nc.vector.tensor_copy(
                    ts2[rank : 2 * rank, fh * f_chunk : (fh + 1) * f_chunk],
                    pt[rank : 2 * rank, f * f_chunk : (f + 1) * f_chunk],
                )
            nc.sync.dma_start(t_sb[0:rank, sh:s], ts2[rank : 2 * rank, 0 : s - sh])
            delta_and_drain(ps0, 0)

            for mt in range(1, mt_n):
                w_sb = wpool.tile([P, kt_n * P], bf, tag="w")
                nc.sync.dma_start(w_sb[:], w[mt, :, :])
                ps = mpool.tile([P, s], f32, tag="ps")
                for kt in range(kt_n):
                    k_loop(ps, w_sb, kt)
                delta_and_drain(ps, mt)

    nc.compile()
    return nc


def _get_nc(s=S, in_f=IN, out_f=OUT, rank=RANK, f_chunk=512):
    key = (s, in_f, out_f, rank, f_chunk)
    if key not in _NC_CACHE:
        _NC_CACHE[key] = _build(*key)
    return _NC_CACHE[key]


def _prep_in_maps(x, ids, weight, bias, lora_a, lora_b):
    """Host-side shard + tile + bf16 cast. Returns one in_map per core."""
    bf16 = ml_dtypes.bfloat16
    s, in_f = x.shape[1], x.shape[2]
    out_f = weight.shape[0]
    rank = lora_a.shape[2]
    kt_n = in_f // P
    mt_n = out_f // P

    # weight -> [mt, kp, kt, mj] bf16 (lhsT panels, contiguous per partition)
    wt = (
        weight.reshape(mt_n, P, kt_n, P)
        .transpose(0, 3, 2, 1)
        .astype(bf16)
        .reshape(mt_n, P, kt_n * P)
    )
    wt = np.ascontiguousarray(wt)

    in_maps = []
    for b in range(x.shape[0]):
        xt = (
            x[b]
            .reshape(s, kt_n, P)
            .transpose(2, 1, 0)
            .astype(bf16)
            .reshape(P, kt_n * s)
        )
        at = (
            lora_a[ids[b]]
            .reshape(kt_n, P, rank)
            .transpose(1, 0, 2)
            .astype(bf16)
            .reshape(P, kt_n * rank)
        )
        baug = np.concatenate([lora_b[ids[b]], bias[None, :]], axis=0).astype(bf16)
        in_maps.append(
            {
                "xt": np.ascontiguousarray(xt),
                "w": wt,
                "a": np.ascontiguousarray(at),
                "baug": np.ascontiguousarray(baug),
            }
        )
    return in_maps


def _install_ntff_hook():
    """Provide antenv.axon_hooks so run_bass_kernel_spmd(trace=True) can ship
    NTFF profiles back through the axon tunnel (missing in this image)."""
    import sys
    import types

    if "antenv.axon_hooks" in sys.modules:
        return
    from trn_agent_boot.trn_boot import _ntff_profile_via_ctypes

    hook = _ntff_profile_via_ctypes("/opt/axon/libaxon_pjrt.so")
    if hook is None:
        raise RuntimeError("no profiling symbols in libaxon_pjrt.so")
    mod = types.ModuleType("antenv.axon_hooks")
    mod.get_axon_ntff_profile_hook = lambda: hook
    mod.set_axon_ntff_profile_hook = lambda h: None
    sys.modules["antenv.axon_hooks"] = mod


def kernel(x, lora_int_id, weight, bias, lora_a_weights, lora_b_weights):
    global LAST_EXEC_TIME_NS
    from concourse.bass_utils import run_bass_kernel_spmd

    x = np.asarray(x, dtype=np.float32)
    ids = np.asarray(lora_int_id).astype(np.int64)
    weight = np.asarray(weight, dtype=np.float32)
    bias = np.asarray(bias, dtype=np.float32)
    lora_a = np.asarray(lora_a_weights, dtype=np.float32)
    lora_b = np.asarray(lora_b_weights, dtype=np.float32)

    nc = _get_nc()
    in_maps = _prep_in_maps(x, ids, weight, bias, lora_a, lora_b)

    trace = bool(int(os.environ.get("KERNEL_TRACE", "0")))
    if trace:
        try:
            _install_ntff_hook()
        except Exception:
            trace = False

    res = run_bass_kernel_spmd(nc, in_maps, core_ids=list(range(N_CORES)), trace=trace)
    LAST_EXEC_TIME_NS = res.exec_time_ns
    global LAST_RESULT
    LAST_RESULT = res

    out = np.empty((x.shape[0], x.shape[1], weight.shape[0]), dtype=np.float32)
    for b in range(x.shape[0]):
        out[b] = res.results[b]["out_t"].T
    return out


# revision 14
# speedup vs baseline: 1.0425x; 1.0063x over previous
"""LoRALinear Trainium2 kernel.

out[b] = x[b] @ W.T + bias + (x[b] @ A[id_b]) @ B[id_b]

Strategy (8 NeuronCores, data-parallel over batch):
- Core b handles batch b (2048 tokens). The per-batch LoRA adapter is
  gathered on the host (lora_int_id is host-visible), so each core only
  reads its own A/B stack.
- All matmul operands are pre-cast to bf16 and pre-tiled on the host into
  DMA-friendly layouts (contiguous >=4KB runs per SBUF partition).
- Kernel computes out^T[OUT, S] weight-stationary: for each 128-row output
  tile, accumulate 32 K-tiles of W @ x^T into PSUM (f32), then fold the
  LoRA delta AND the bias in as one extra K=65 accumulation step using
  [B_sel; bias]^T @ [temp^T; ones] before draining PSUM to SBUF/DRAM.
- temp^T = A_sel^T @ x^T is computed once per core in a small pre-pass.
- Host transposes each core's out^T back to [S, OUT].
"""

import os

import numpy as np
import ml_dtypes

P = 128
B, S, IN, OUT = 8, 2048, 4096, 4096
RANK = 64
N_CORES = 8

_NC_CACHE = {}
LAST_EXEC_TIME_NS = None
LAST_RESULT = None


def _build(s, in_f, out_f, rank, f_chunk):
    """Build + compile the single-core Bass program (same NEFF for all cores)."""
    import concourse.tile as tile
    from concourse import bacc, mybir

    kt_n = in_f // P
    mt_n = out_f // P
    nf = s // f_chunk
    bf = mybir.dt.bfloat16
    f32 = mybir.dt.float32

    nc = bacc.Bacc("TRN2", target_bir_lowering=False, debug=False)
    xt = nc.dram_tensor("xt", [P, kt_n * s], bf, kind="ExternalInput")
    w = nc.dram_tensor("w", [mt_n, P, kt_n * P], bf, kind="ExternalInput")
    a = nc.dram_tensor("a", [P, kt_n * rank], bf, kind="ExternalInput")
    baug = nc.dram_tensor("baug", [rank + 1, out_f], bf, kind="ExternalInput")
    out_t = nc.dram_tensor("out_t", [out_f, s], f32, kind="ExternalOutput")

    with tile.TileContext(nc) as tc:
        with (
            tc.tile_pool(name="xp", bufs=1) as xp,
            tc.tile_pool(name="cst", bufs=1) as cst,
            tc.tile_pool(name="wp", bufs=3) as wpool,
            tc.tile_pool(name="op", bufs=2) as opool,
            tc.tile_pool(name="mp", bufs=2, space="PSUM") as mpool,
        ):
            # DMA order on the sync (SP-HWDGE) queue: small constants + first
            # weight panel BEFORE the big x stream, so PE can start ~6us in
            # and trail the x DMA chunk-by-chunk. Output DMAs go on the
            # scalar (ACT-HWDGE) queue so they never block weight prefetch.
            a_sb = cst.tile([P, kt_n * rank], bf)
            nc.sync.dma_start(a_sb[:], a[:])
            x_sb = xp.tile([P, kt_n * s], bf)
            b_sb = cst.tile([rank + 1, out_f], bf)
            w_sb0 = wpool.tile([P, kt_n * P], bf, tag="w")
            for kt in range(kt_n):
                nc.sync.dma_start(x_sb[:, kt * s : (kt + 1) * s], xt[:, kt * s : (kt + 1) * s])
                if kt == 0:
                    # first weight panel right after the first x chunk: PE can
                    # start at ~9us and trail the x stream chunk-by-chunk
                    nc.sync.dma_start(w_sb0[:], w[0, :, :])
                elif kt == min(8, kt_n - 1):
                    # B/bias panel not needed until the first delta (~70us)
                    nc.sync.dma_start(b_sb[:], baug[:])
            t_sb = cst.tile([rank + 1, s], bf)
            nc.vector.memset(t_sb[rank : rank + 1, :], 1.0)
            ts2 = cst.tile([2 * rank, s - (nf // 2) * f_chunk], bf)

            def k_loop(ps, w_sb, kt, stop_last=False):
                for f in range(nf):
                    nc.tensor.matmul(
                        ps[:, f * f_chunk : (f + 1) * f_chunk],
                        w_sb[:, kt * P : (kt + 1) * P],
                        x_sb[:, kt * s + f * f_chunk : kt * s + (f + 1) * f_chunk],
                        start=(kt == 0),
                        stop=stop_last and (kt == kt_n - 1),
                    )

            def delta_and_drain(ps, mt):
                # LoRA delta + bias: one K=rank+1 accumulation step, then
                # drain + store per f-chunk so the copy/DMA of chunk f
                # overlaps the delta matmul of chunk f+1.
                ob = opool.tile([P, s], f32, tag="ob")
                for f in range(nf):
                    fs = slice(f * f_chunk, (f + 1) * f_chunk)
                    nc.tensor.matmul(
                        ps[:, fs],
                        b_sb[:, mt * P : (mt + 1) * P],
                        t_sb[:, fs],
                        start=False,
                        stop=True,
                    )
                    nc.vector.tensor_copy(ob[:, fs], ps[:, fs])
                    nc.scalar.dma_start(out_t[mt * P : (mt + 1) * P, fs], ob[:, fs])

            # ramp: pre-pass (temp^T = A^T @ x^T) interleaved with mt=0's
            # k-loop, both trailing the x DMA chunk-by-chunk. The pre-pass
            # is col-packed: M=64 uses only half the PE column groups, so
            # token-half 0 runs on PSUM partitions 0:64 and token-half 1 on
            # 64:128 concurrently (distinct col groups, distinct banks).
            assert nf % 2 == 0 and 2 * rank == P
            nh = nf // 2
            sh = nh * f_chunk
            pt = mpool.tile([2 * rank, s], f32, tag="ps")
            ps0 = mpool.tile([P, s], f32, tag="ps")
            for kt in range(kt_n):
                for fh in range(nh):
                    for half, f in ((0, fh), (1, fh + nh)):
                        nc.tensor.matmul(
                            pt[half * rank : (half + 1) * rank, f * f_chunk : (f + 1) * f_chunk],
                            a_sb[:, kt * rank : (kt + 1) * rank],
                            x_sb[:, kt * s + f * f_chunk : kt * s + (f + 1) * f_chunk],
                            start=(kt == 0),
                            stop=(kt == kt_n - 1),
                        )
                k_loop(ps0, w_sb0, kt)
            # assemble temp^T (+ones row already set) on partitions 0:64;
            # token-half 1 lands on partitions 64:128 so it bounces through
            # ts2 + an SBUF->SBUF DMA (engines can't move across partitions).
            for fh in range(nh):
                f = fh + nh
                nc.vector.tensor_copy(
                    t_sb[0:rank, fh * f_chunk : (fh + 1) * f_chunk],
                    pt[0:rank, fh * f_chunk : (fh + 1) * f_chunk],
                )
                nc.vector.tensor_copy(
                    ts2[rank : 2 * rank, fh * f_chunk : (fh + 1) * f_chunk],
                    pt[rank : 2 * rank, f * f_chunk : (f + 1) * f_chunk],
                )
            nc.sync.dma_start(t_sb[0:rank, sh:s], ts2[rank : 2 * rank, 0 : s - sh])

            # mt=1's k-loop runs while the Taug assembly (DVE+DMA) completes;
            # mt=0's delta+drain slots in right after, then steady state.
            w_sb1 = wpool.tile([P, kt_n * P], bf, tag="w")
            nc.sync.dma_start(w_sb1[:], w[1, :, :])
            ps1 = mpool.tile([P, s], f32, tag="ps")
            for kt in range(kt_n):
                k_loop(ps1, w_sb1, kt)
            delta_and_drain(ps0, 0)
            delta_and_drain(ps1, 1)

            for mt in range(2, mt_n):
                w_sb = wpool.tile([P, kt_n * P], bf, tag="w")
                nc.sync.dma_start(w_sb[:], w[mt, :, :])
                ps = mpool.tile([P, s], f32, tag="ps")
                for kt in range(kt_n):
                    k_loop(ps, w_sb, kt)
                delta_and_drain(ps, mt)

    nc.compile()
    return nc


def _get_nc(s=S, in_f=IN, out_f=OUT, rank=RANK, f_chunk=512):
    key = (s, in_f, out_f, rank, f_chunk)
    if key not in _NC_CACHE:
        _NC_CACHE[key] = _build(*key)
    return _NC_CACHE[key]


def _prep_in_maps(x, ids, weight, bias, lora_a, lora_b):
    """Host-side shard + tile + bf16 cast. Returns one in_map per core."""
    bf16 = ml_dtypes.bfloat16
    s, in_f = x.shape[1], x.shape[2]
    out_f = weight.shape[0]
    rank = lora_a.shape[2]
    kt_n = in_f // P
    mt_n = out_f // P

    # weight -> [mt, kp, kt, mj] bf16 (lhsT panels, contiguous per partition)
    wt = (
        weight.reshape(mt_n, P, kt_n, P)
        .transpose(0, 3, 2, 1)
        .astype(bf16)
        .reshape(mt_n, P, kt_n * P)
    )
    wt = np.ascontiguousarray(wt)

    in_maps = []
    for b in range(x.shape[0]):
        xt = (
            x[b]
            .reshape(s, kt_n, P)
            .transpose(2, 1, 0)
            .astype(bf16)
            .reshape(P, kt_n * s)
        )
        at = (
            lora_a[ids[b]]
            .reshape(kt_n, P, rank)
            .transpose(1, 0, 2)
            .astype(bf16)
            .reshape(P, kt_n * rank)
        )
        baug = np.concatenate([lora_b[ids[b]], bias[None, :]], axis=0).astype(bf16)
        in_maps.append(
            {
                "xt": np.ascontiguousarray(xt),
                "w": wt,
                "a": np.ascontiguousarray(at),
                "baug": np.ascontiguousarray(baug),
            }
        )
    return in_maps


def _install_ntff_hook():
    """Provide antenv.axon_hooks so run_bass_kernel_spmd(trace=True) can ship
    NTFF profiles back through the axon tunnel (missing in this image)."""
    import sys
    import types

    if "antenv.axon_hooks" in sys.modules:
        return
    from trn_agent_boot.trn_boot import _ntff_profile_via_ctypes

    hook = _ntff_profile_via_ctypes("/opt/axon/libaxon_pjrt.so")
    if hook is None:
        raise RuntimeError("no profiling symbols in libaxon_pjrt.so")
    mod = types.ModuleType("antenv.axon_hooks")
    mod.get_axon_ntff_profile_hook = lambda: hook
    mod.set_axon_ntff_profile_hook = lambda h: None
    sys.modules["antenv.axon_hooks"] = mod


def kernel(x, lora_int_id, weight, bias, lora_a_weights, lora_b_weights):
    global LAST_EXEC_TIME_NS
    from concourse.bass_utils import run_bass_kernel_spmd

    x = np.asarray(x, dtype=np.float32)
    ids = np.asarray(lora_int_id).astype(np.int64)
    weight = np.asarray(weight, dtype=np.float32)
    bias = np.asarray(bias, dtype=np.float32)
    lora_a = np.asarray(lora_a_weights, dtype=np.float32)
    lora_b = np.asarray(lora_b_weights, dtype=np.float32)

    nc = _get_nc()
    in_maps = _prep_in_maps(x, ids, weight, bias, lora_a, lora_b)

    trace = bool(int(os.environ.get("KERNEL_TRACE", "0")))
    if trace:
        try:
            _install_ntff_hook()
        except Exception:
            trace = False

    res = run_bass_kernel_spmd(nc, in_maps, core_ids=list(range(N_CORES)), trace=trace)
    LAST_EXEC_TIME_NS = res.exec_time_ns
    global LAST_RESULT
    LAST_RESULT = res

    out = np.empty((x.shape[0], x.shape[1], weight.shape[0]), dtype=np.float32)
    for b in range(x.shape[0]):
        out[b] = res.results[b]["out_t"].T
    return out
